# revision 2
# baseline (speedup 1.0000x reference)
"""BitLinear (per-token int8 activation quant + ternary weight quant + matmul)
as a Bass/Tile kernel on 8 Trainium2 NeuronCores.

Strategy (data-parallel tokens / sharded weight prep):
  - x [4,2048,4096] -> [8192,4096]; each core gets a 1024-token slab.
  - weight [4096,4096]; core i ternarizes out_feature rows [512i, 512(i+1));
    global mean(|W|) via tiny AllReduce; ternarized slabs shared via
    AllGather (bf16, exact), chunked 4x along the contraction dim so the
    matmul pipeline starts as soon as the first chunk lands.
  - q = rint(x*s) (s = 127/max(|x|) per token) and tw in {-1,0,1} are exact
    in bf16, so the bf16 matmul with fp32 PSUM accumulation is EXACT integer
    arithmetic; dequant scales are applied on the PSUM->SBUF copy.
  - Operand transposes (contraction dim must be on partitions) run on the
    DMA xbar transpose engine, keeping the PE free for matmuls.
  - DMA queue split: W-prep chain on nc.scalar (HWDGE#2), x loads + xbar
    transposes on nc.sync (HWDGE#1), output writes on nc.gpsimd (SWDGE).
"""
import numpy as np
from contextlib import ExitStack

N_CORES = 8
B, S, D_IN, D_OUT = 4, 2048, 4096, 4096
TOK = B * S                 # 8192
TOK_PC = TOK // N_CORES     # 1024 tokens per core
OF_PC = D_OUT // N_CORES    # 512 out-features ternarized per core
N_TOK_TILES = TOK_PC // 128  # 8
N_K = D_IN // 128            # 32 contraction tiles
OF_CHUNK = 512
N_SLAB = D_OUT // OF_CHUNK   # 8
N_AG = 4                     # AllGather chunks along D_IN
AG_COLS = D_IN // N_AG       # 1024
K_PER_AG = AG_COLS // 128    # 8 contraction tiles per AG chunk
EPS = 1e-5
MAGIC = float(np.float32(1.5 * 2 ** 23))   # fp32 round-to-nearest-even trick
MEAN_SCALE = float(np.float32(1.0 / (D_IN * D_OUT)))  # 2^-24, exact

_CACHE = {}


def _build_module():
    import concourse.bacc as bacc
    import concourse.tile as tile
    import concourse.mybir as mybir
    import concourse.bass_isa as bass_isa

    dt = mybir.dt
    AF = mybir.ActivationFunctionType
    AL = mybir.AluOpType
    AX = mybir.AxisListType

    nc = bacc.Bacc(
        "TRN2", target_bir_lowering=False, debug=False, num_devices=N_CORES
    )
    xs = nc.dram_tensor("xs", [TOK_PC, D_IN], dt.float32, kind="ExternalInput").ap()
    ws = nc.dram_tensor("ws", [OF_PC, D_IN], dt.float32, kind="ExternalInput").ap()
    out = nc.dram_tensor("out", [TOK_PC, D_OUT], dt.float32, kind="ExternalOutput").ap()

    wsum_d = nc.dram_tensor("wsum_d", [128, 1], dt.float32).ap()
    wsum_sh = nc.dram_tensor("wsum_sh", [128, 1], dt.float32, addr_space="Shared").ap()
    tw_d = [
        nc.dram_tensor(f"tw_d{k}", [OF_PC, AG_COLS], dt.bfloat16).ap()
        for k in range(N_AG)
    ]
    tw_full = [
        nc.dram_tensor(
            f"tw_full{k}", [D_OUT, AG_COLS], dt.bfloat16, addr_space="Shared"
        ).ap()
        for k in range(N_AG)
    ]

    NWT = OF_PC // 128  # 4 weight row-blocks per core

    with tile.TileContext(nc) as tc, ExitStack() as ctx:
        stats = ctx.enter_context(tc.tile_pool(name="stats", bufs=1))
        qT_pool = ctx.enter_context(tc.tile_pool(name="qT", bufs=N_TOK_TILES))
        pp = ctx.enter_context(tc.tile_pool(name="pp", bufs=6, space="PSUM"))

        amc = stats.tile([128, N_TOK_TILES], dt.float32, tag="amc")
        s_all = stats.tile([128, N_TOK_TILES], dt.float32, tag="s_all")
        dq = stats.tile([128, N_TOK_TILES], dt.float32, tag="dq")
        wme = stats.tile([128, 1], dt.float32, tag="wme")
        swt = stats.tile([128, 1], dt.float32, tag="swt")
        wp = stats.tile([128, NWT], dt.float32, tag="wp")
        wsum_sb = stats.tile([128, 1], dt.float32, tag="wsum_sb")
        gsb = stats.tile([128, 1], dt.float32, tag="gsb")
        gtot = stats.tile([128, 1], dt.float32, tag="gtot")

        qT_tiles = []
        with (
            tc.tile_pool(name="wpool", bufs=3) as wpool,
            tc.tile_pool(name="xpool", bufs=2) as xpool,
            tc.tile_pool(name="qp", bufs=1) as qp,
            tc.tile_pool(name="twp", bufs=4) as twp,
        ):
            # ---- W slab |W| partial sums -> AllReduce -> scales ----
            with nc.named_scope("wsum"):
                for j in range(NWT):
                    wt = wpool.tile([128, D_IN], dt.float32, tag="w", name=f"wt{j}")
                    nc.scalar.dma_start(wt[:], ws[j * 128:(j + 1) * 128, :])
                    nc.vector.tensor_reduce(
                        wp[:, j:j + 1], wt[:], axis=AX.X, op=AL.add,
                        apply_absolute_value=True,
                    )
                nc.vector.tensor_reduce(wsum_sb[:], wp[:], axis=AX.X, op=AL.add)
                nc.scalar.dma_start(wsum_d[:], wsum_sb[:])
                nc.gpsimd.collective_compute(
                    "AllReduce", AL.add,
                    replica_groups=[list(range(N_CORES))],
                    ins=[wsum_d[:]], outs=[wsum_sh[:]],
                )
                nc.scalar.dma_start(gsb[:], wsum_sh[:])
                nc.gpsimd.partition_all_reduce(
                    gtot[:], gsb[:], channels=128, reduce_op=bass_isa.ReduceOp.add
                )
                nc.vector.tensor_scalar(
                    wme[:], gtot[:], MEAN_SCALE, EPS, op0=AL.mult, op1=AL.max
                )
                nc.vector.reciprocal(swt[:], wme[:])

            # ---- ternarize own W slab (re-reads prefetch early on scalar q) ----
            tw_tiles = []
            with nc.named_scope("terniarize"):
                for j in range(NWT):
                    wt2 = wpool.tile([128, D_IN], dt.float32, tag="w", name=f"wt2_{j}")
                    nc.scalar.dma_start(wt2[:], ws[j * 128:(j + 1) * 128, :])
                    wsc = wpool.tile([128, D_IN], dt.float32, tag="w", name=f"wsc{j}")
                    nc.scalar.activation(wsc[:], wt2[:], AF.Copy, scale=swt[:, 0:1])
                    twr = qp.tile([128, D_IN], dt.bfloat16, tag="qb", name=f"twr{j}")
                    nc.vector.tensor_scalar(
                        twr[:], wsc[:], MAGIC, MAGIC, op0=AL.add, op1=AL.subtract
                    )
                    twc = twp.tile([128, D_IN], dt.bfloat16, tag="twc", name=f"twc{j}")
                    nc.vector.tensor_scalar(
                        twc[:], twr[:], 1.0, -1.0, op0=AL.min, op1=AL.max
                    )
                    tw_tiles.append(twc)
                # write chunk-major so each AllGather chunk fires asap
                for k in range(N_AG):
                    for j in range(NWT):
                        nc.scalar.dma_start(
                            tw_d[k][j * 128:(j + 1) * 128, :],
                            tw_tiles[j][:, k * AG_COLS:(k + 1) * AG_COLS],
                        )
                    nc.gpsimd.collective_compute(
                        "AllGather", AL.bypass,
                        replica_groups=[list(range(N_CORES))],
                        ins=[tw_d[k][:]], outs=[tw_full[k][:]],
                    )

            # ---- per-token activation quant + xbar transpose ----
            with nc.named_scope("xquant"):
                for t in range(N_TOK_TILES):
                    xt = xpool.tile([128, D_IN], dt.float32, tag="x", name=f"xt{t}")
                    nc.sync.dma_start(xt[:], xs[t * 128:(t + 1) * 128, :])
                    nc.vector.tensor_reduce(
                        amc[:, t:t + 1], xt[:], axis=AX.X, op=AL.max,
                        apply_absolute_value=True,
                    )
                    nc.vector.tensor_scalar(
                        amc[:, t:t + 1], amc[:, t:t + 1], EPS, None, op0=AL.max
                    )
                    nc.vector.reciprocal(s_all[:, t:t + 1], amc[:, t:t + 1])
                    nc.vector.tensor_scalar(
                        s_all[:, t:t + 1], s_all[:, t:t + 1], 127.0, None,
                        op0=AL.mult,
                    )
                    xsc = xpool.tile([128, D_IN], dt.float32, tag="x", name=f"xsc{t}")
                    nc.scalar.activation(
                        xsc[:], xt[:], AF.Copy, scale=s_all[:, t:t + 1]
                    )
                    qb = qp.tile([128, D_IN], dt.bfloat16, tag="qb", name=f"qb{t}")
                    nc.vector.tensor_scalar(
                        qb[:], xsc[:], MAGIC, MAGIC, op0=AL.add, op1=AL.subtract
                    )
                    qT_t = qT_pool.tile(
                        [128, N_K, 128], dt.bfloat16, tag="qT", name=f"qT{t}"
                    )
                    nc.sync.dma_start(qT_t[:], qb[:], transpose=True)
                    qT_tiles.append(qT_t)
                # dq = amax_c * mean_c / 127 (after wme is ready)
                for t in range(N_TOK_TILES):
                    nc.vector.tensor_scalar(
                        dq[:, t:t + 1], amc[:, t:t + 1], wme[:, 0:1],
                        float(np.float32(1.0 / 127.0)), op0=AL.mult, op1=AL.mult,
                    )

        # ---- matmul + dequant ----
        with (
            tc.tile_pool(name="twT", bufs=2 * N_AG) as twTp,
            tc.tile_pool(name="op", bufs=3) as op,
        ):
            with nc.named_scope("matmul"):
                for c in range(N_SLAB):
                    octs = []
                    for g in range(N_AG):
                        twT_o = twTp.tile(
                            [128, K_PER_AG, OF_CHUNK], dt.bfloat16, tag="twT",
                            name=f"twT{c}_{g}",
                        )
                        nc.sync.dma_start(
                            twT_o[:],
                            tw_full[g][c * OF_CHUNK:(c + 1) * OF_CHUNK, :],
                            transpose=True,
                        )
                        octs.append(twT_o)
                    for t in range(N_TOK_TILES):
                        ps = pp.tile(
                            [128, OF_CHUNK], dt.float32, tag="ps", name=f"ps{c}_{t}"
                        )
                        for k in range(N_K):
                            nc.tensor.matmul(
                                ps[:], qT_tiles[t][:, k, :],
                                octs[k // K_PER_AG][:, k % K_PER_AG, :],
                                start=(k == 0), stop=(k == N_K - 1),
                            )
                        ot = op.tile(
                            [128, OF_CHUNK], dt.float32, tag="ot", name=f"ot{c}_{t}"
                        )
                        nc.scalar.mul(ot[:], ps[:], dq[:, t:t + 1])
                        nc.gpsimd.dma_start(
                            out[t * 128:(t + 1) * 128,
                                c * OF_CHUNK:(c + 1) * OF_CHUNK],
                            ot[:],
                        )

    nc.compile()
    return nc


def _get_module():
    if "nc" not in _CACHE:
        _CACHE["nc"] = _build_module()
    return _CACHE["nc"]


def kernel(x: np.ndarray, weight: np.ndarray) -> np.ndarray:
    from concourse.bass_utils import run_bass_kernel_spmd

    x = np.asarray(x, dtype=np.float32)
    weight = np.asarray(weight, dtype=np.float32)
    x2 = np.ascontiguousarray(x.reshape(TOK, D_IN))
    w2 = np.ascontiguousarray(weight)

    in_maps = [
        {
            "xs": x2[i * TOK_PC:(i + 1) * TOK_PC],
            "ws": w2[i * OF_PC:(i + 1) * OF_PC],
        }
        for i in range(N_CORES)
    ]
    nc = _get_module()
    res = run_bass_kernel_spmd(nc, in_maps, list(range(N_CORES)))
    out = np.concatenate([res.results[i]["out"] for i in range(N_CORES)], axis=0)
    return out.reshape(B, S, D_OUT)
